# revision 6
# baseline (speedup 1.0000x reference)
"""Trainium2 Bass kernel for the 2-class Gaussian prior log-prob loss.

Reference math (N=8192 samples, D=3072 dims, 2 balanced classes):
    mus[c]  = mean over rows of class c of `mean`
    lsds[c] = mean over rows of class c of `log_sd`
    logp[n] = sum_d [ -0.5*log(2pi) - lsds[t_n,d]
                      - 0.5*(z[n,d]-mus[t_n,d])^2 * exp(-2*lsds[t_n,d]) ]
    log_p_total[c] = class-mean of logp;  prior = mean(log_p_total + logdet_total)

Distribution: data-parallel over N across 8 NeuronCores (1024 rows/core).
Per-class sums of mean/log_sd are computed on the TensorEngine (one-hot
stationary matmul), combined with a single [4,3072] AllReduce, then the
per-sample quadratic term is fully local:
    v[n,d]  = (z[n,d] - mus[t_n,d]) * sqrt(0.5*exp(-2*lsds[t_n,d]))
    rq[n]   = sum_d v^2      (ScalarE Square + accumulate)
    logp[n] = C2[t_n] - rq[n],  C2[c] = -0.5*log(2pi)*D - sum_d lsds[c,d]
The tiny class-level reductions (log_p_total, logdet_total, prior) are host-side.
"""

import numpy as np

import concourse.bass as bass
import concourse.bacc as bacc
import concourse.mybir as mybir
import concourse.tile as tile
from concourse.bass_utils import run_bass_kernel_spmd

LOG_2PI = float(np.log(2.0 * np.pi))

N_CORES = 8
N = 8192
D = 3072
NL = N // N_CORES          # rows per core
P = 128                    # SBUF partitions
T = NL // P                # row tiles per core
CH = 512                   # matmul free-dim chunk (one PSUM bank of fp32)
NCH = D // CH

F32 = mybir.dt.float32

# Set by test.py to capture a hardware trace; last BassKernelResults lands in
# LAST_RESULTS for exec-time inspection.
TRACE = False
LAST_RESULTS = None

_CACHED_NC = None


def _build_nc():
    nc = bacc.Bacc(
        "TRN2",
        target_bir_lowering=False,
        debug=False,
        num_devices=N_CORES,
    )

    z_d = nc.dram_tensor("z", [NL, D], F32, kind="ExternalInput").ap()
    mean_d = nc.dram_tensor("mean", [NL, D], F32, kind="ExternalInput").ap()
    lsd_d = nc.dram_tensor("lsd", [NL, D], F32, kind="ExternalInput").ap()
    oh_d = nc.dram_tensor("oh", [P, 2], F32, kind="ExternalInput").ap()
    selT_d = nc.dram_tensor("selT", [2, P], F32, kind="ExternalInput").ap()
    invc_d = nc.dram_tensor("invc", [2, 1], F32, kind="ExternalInput").ap()

    rq_d = nc.dram_tensor("rq", [T, P], F32, kind="ExternalOutput").ap()
    mus_d = nc.dram_tensor("mus", [2, D], F32, kind="ExternalOutput").ap()
    lsds_d = nc.dram_tensor("lsds", [2, D], F32, kind="ExternalOutput").ap()

    with tile.TileContext(nc) as tc:
        with (
            tc.tile_pool(name="consts", bufs=1) as cp,
            tc.tile_pool(name="stream", bufs=2) as sp,
            tc.tile_pool(name="zpool", bufs=3) as zp,
            tc.tile_pool(name="coef", bufs=4) as cfp,
            tc.tile_pool(name="work", bufs=2) as wp,
            tc.tile_pool(name="acc", bufs=1, space="PSUM") as pp,
            tc.tile_pool(name="bcast", bufs=2, space="PSUM") as pbp,
            tc.tile_pool(name="dram", bufs=1, space="DRAM") as dp,
        ):
            oh = cp.tile([P, 2], F32)
            nc.sync.dma_start(oh, oh_d)
            selT = cp.tile([2, P], F32)
            nc.sync.dma_start(selT, selT_d)
            invc = cp.tile([2, 1], F32)
            nc.sync.dma_start(invc, invc_d)

            # ---- phase 1: per-class partial sums of mean / log_sd ----
            # PE output base partition must be 0/32/64: mean sums live at
            # partitions 0-1, log_sd sums at partitions 32-33.
            ps = pp.tile([34, D], F32)
            for t in range(T):
                mt = sp.tile([P, D], F32, tag="m", name=f"m{t}")
                nc.sync.dma_start(mt, mean_d[t * P:(t + 1) * P, :])
                lt = sp.tile([P, D], F32, tag="l", name=f"l{t}")
                nc.sync.dma_start(lt, lsd_d[t * P:(t + 1) * P, :])
                for j in range(NCH):
                    cs = slice(j * CH, (j + 1) * CH)
                    nc.tensor.matmul(
                        ps[0:2, cs], oh, mt[:, cs],
                        start=(t == 0), stop=(t == T - 1),
                    )
                    nc.tensor.matmul(
                        ps[32:34, cs], oh, lt[:, cs],
                        start=(t == 0), stop=(t == T - 1),
                    )

            # ---- all-reduce the partials across the 8 cores ----
            # cc layout [2, 2*D]: row c = [sum_mean_c (D) | sum_lsd_c (D)].
            # PSUM partials bounce through small SBUF tiles into DRAM.
            cc_in = dp.tile([2, 2 * D], F32)
            cc_out = dp.tile([2, 2 * D], F32, addr_space="Shared")
            Sm = cfp.tile([2, D], F32, tag="c", name="Sm")
            nc.scalar.copy(Sm, ps[0:2, :])
            nc.sync.dma_start(cc_in[:, 0:D], Sm)
            Sl = cfp.tile([2, D], F32, tag="c", name="Sl")
            nc.scalar.copy(Sl, ps[32:34, :])
            nc.sync.dma_start(cc_in[:, D:2 * D], Sl)
            nc.gpsimd.collective_compute(
                "AllReduce",
                mybir.AluOpType.add,
                replica_groups=[list(range(N_CORES))],
                ins=[cc_in.opt()],
                outs=[cc_out.opt()],
            )
            Gm = cfp.tile([2, D], F32, tag="c", name="Gm")
            nc.sync.dma_start(Gm, cc_out[:, 0:D])
            Gl = cfp.tile([2, D], F32, tag="c", name="Gl")
            nc.sync.dma_start(Gl, cc_out[:, D:2 * D])

            # ---- coefficients ----
            Mm = cfp.tile([2, D], F32, tag="c", name="Mm")  # mus
            nc.vector.tensor_scalar_mul(Mm, Gm, invc)
            nc.sync.dma_start(mus_d, Mm)
            Ml = cfp.tile([2, D], F32, tag="c", name="Ml")  # lsds
            nc.vector.tensor_scalar_mul(Ml, Gl, invc)
            nc.sync.dma_start(lsds_d, Ml)
            IV = cfp.tile([2, D], F32, tag="c", name="IV")  # exp(-2*lsd)
            nc.scalar.activation(
                IV, Ml, mybir.ActivationFunctionType.Exp, scale=-2.0
            )
            SB = cfp.tile([2, D], F32, tag="c", name="SB")  # sqrt(0.5*exp(-2*lsd))
            nc.scalar.activation(
                SB, IV, mybir.ActivationFunctionType.Sqrt, scale=0.5
            )

            # Broadcast per-class rows to the 128-row class pattern via PE:
            # out[p, :] = coef[class(p), :]   (selT is the one-hot pattern^T)
            MU_bc = cp.tile([P, D], F32)
            SB_bc = cp.tile([P, D], F32)
            for j in range(NCH):
                cs = slice(j * CH, (j + 1) * CH)
                pm = pbp.tile([P, CH], F32, tag="bc", name=f"pm{j}")
                nc.tensor.matmul(pm, selT, Mm[:, cs], start=True, stop=True)
                nc.scalar.copy(MU_bc[:, cs], pm)
                pb = pbp.tile([P, CH], F32, tag="bc", name=f"pb{j}")
                nc.tensor.matmul(pb, selT, SB[:, cs], start=True, stop=True)
                nc.scalar.copy(SB_bc[:, cs], pb)

            # ---- phase 2: rq[n] = sum_d ((z - mu_sel) * sb_sel)^2 ----
            for t in range(T):
                zt = zp.tile([P, D], F32, tag="z", name=f"z{t}")
                nc.scalar.dma_start(zt, z_d[t * P:(t + 1) * P, :])
                u = wp.tile([P, D], F32, tag="u", name=f"u{t}")
                nc.vector.tensor_sub(u, zt, MU_bc)
                nc.vector.tensor_mul(u, u, SB_bc)
                v2 = wp.tile([P, D], F32, tag="v2", bufs=1, name=f"v2_{t}")
                rq = wp.tile([P, 1], F32, tag="rq", name=f"rq{t}")
                nc.scalar.activation(
                    v2, u, mybir.ActivationFunctionType.Square, accum_out=rq
                )
                nc.sync.dma_start(rq_d[t, :], rq[:, 0])

    nc.compile()
    return nc


def kernel(z, mean, log_sd, logdet, target):
    global LAST_RESULTS, _CACHED_NC

    z = np.ascontiguousarray(np.asarray(z, dtype=np.float32))
    mean = np.ascontiguousarray(np.asarray(mean, dtype=np.float32))
    log_sd = np.ascontiguousarray(np.asarray(log_sd, dtype=np.float32))
    logdet64 = np.asarray(logdet, dtype=np.float64)
    tgt = np.asarray(target).astype(np.int64)
    n, d = z.shape
    assert (n, d) == (N, D), f"kernel hardcoded for {(N, D)}, got {(n, d)}"

    # The device kernel assumes every 128-row tile has the same class pattern
    # (true for the arange%2 labels). Otherwise interleave the (balanced)
    # classes host-side and un-permute logp afterwards.
    pat = tgt[:P]
    perm = None
    tgt_dev = tgt
    if not (tgt.reshape(-1, P) == pat[None, :]).all():
        idx0 = np.where(tgt == 0)[0]
        idx1 = np.where(tgt == 1)[0]
        assert len(idx0) == len(idx1), "fallback layout needs balanced classes"
        perm = np.empty(n, dtype=np.int64)
        perm[0::2] = idx0
        perm[1::2] = idx1
        z, mean, log_sd = z[perm], mean[perm], log_sd[perm]
        tgt_dev = tgt[perm]
        pat = tgt_dev[:P]

    counts = np.array([(tgt == 0).sum(), (tgt == 1).sum()], dtype=np.float64)
    patf = pat.astype(np.float32)
    oh_np = np.ascontiguousarray(np.stack([1.0 - patf, patf], axis=1))  # [P, 2]
    selT_np = np.ascontiguousarray(oh_np.T)  # [2, P]
    invc_np = (1.0 / counts).astype(np.float32).reshape(2, 1)

    if _CACHED_NC is None:
        _CACHED_NC = _build_nc()
    nc = _CACHED_NC

    in_maps = [
        {
            "z": z[i * NL:(i + 1) * NL],
            "mean": mean[i * NL:(i + 1) * NL],
            "lsd": log_sd[i * NL:(i + 1) * NL],
            "oh": oh_np,
            "selT": selT_np,
            "invc": invc_np,
        }
        for i in range(N_CORES)
    ]
    res = run_bass_kernel_spmd(
        nc, in_maps, core_ids=list(range(N_CORES)), trace=TRACE
    )
    LAST_RESULTS = res

    rq = np.concatenate(
        [np.asarray(res.results[i]["rq"]).reshape(-1) for i in range(N_CORES)]
    )
    mus = np.asarray(res.results[0]["mus"])
    lsds = np.asarray(res.results[0]["lsds"])

    # logp[n] = C2[t_n] - rq[n];  C2[c] = -0.5*log(2pi)*D - sum_d lsds[c, d]
    c2 = -0.5 * LOG_2PI * D - lsds.astype(np.float64).sum(axis=1)
    logp = (c2[tgt_dev] - rq.astype(np.float64)).astype(np.float32)
    if perm is not None:
        inv = np.empty_like(perm)
        inv[perm] = np.arange(n)
        logp = logp[inv]

    logp64 = logp.astype(np.float64)
    lp_tot = np.array(
        [logp64[tgt == 0].sum() / counts[0], logp64[tgt == 1].sum() / counts[1]]
    )
    ld_tot = np.array(
        [logdet64[tgt == 0].sum() / counts[0], logdet64[tgt == 1].sum() / counts[1]]
    )
    prior_logprob = np.float32((lp_tot + ld_tot).mean())
    log_p_total = lp_tot.astype(np.float32)

    return prior_logprob, mus, lsds, logp, log_p_total


# revision 7
# speedup vs baseline: 1.0283x; 1.0283x over previous
"""Trainium2 Bass kernel for the 2-class Gaussian prior log-prob loss.

Reference math (N=8192 samples, D=3072 dims, 2 balanced classes):
    mus[c]  = mean over rows of class c of `mean`
    lsds[c] = mean over rows of class c of `log_sd`
    logp[n] = sum_d [ -0.5*log(2pi) - lsds[t_n,d]
                      - 0.5*(z[n,d]-mus[t_n,d])^2 * exp(-2*lsds[t_n,d]) ]
    log_p_total[c] = class-mean of logp;  prior = mean(log_p_total + logdet_total)

Distribution: data-parallel over N across 8 NeuronCores (1024 rows/core).
Per-class sums of mean/log_sd are computed on the TensorEngine (one-hot
stationary matmul), combined with a single [4,3072] AllReduce, then the
per-sample quadratic term is fully local:
    v[n,d]  = (z[n,d] - mus[t_n,d]) * sqrt(0.5*exp(-2*lsds[t_n,d]))
    rq[n]   = sum_d v^2      (ScalarE Square + accumulate)
    logp[n] = C2[t_n] - rq[n],  C2[c] = -0.5*log(2pi)*D - sum_d lsds[c,d]
The tiny class-level reductions (log_p_total, logdet_total, prior) are host-side.
"""

import numpy as np

import concourse.bass as bass
import concourse.bacc as bacc
import concourse.mybir as mybir
import concourse.tile as tile
from concourse.bass_utils import run_bass_kernel_spmd

LOG_2PI = float(np.log(2.0 * np.pi))

N_CORES = 8
N = 8192
D = 3072
NL = N // N_CORES          # rows per core
P = 128                    # SBUF partitions
T = NL // P                # row tiles per core
CH = 512                   # matmul free-dim chunk (one PSUM bank of fp32)
NCH = D // CH

F32 = mybir.dt.float32

# Set by test.py to capture a hardware trace; last BassKernelResults lands in
# LAST_RESULTS for exec-time inspection.
TRACE = False
LAST_RESULTS = None

_CACHED_NC = None


def _build_nc():
    nc = bacc.Bacc(
        "TRN2",
        target_bir_lowering=False,
        debug=False,
        num_devices=N_CORES,
    )

    z_d = nc.dram_tensor("z", [NL, D], F32, kind="ExternalInput").ap()
    mean_d = nc.dram_tensor("mean", [NL, D], F32, kind="ExternalInput").ap()
    lsd_d = nc.dram_tensor("lsd", [NL, D], F32, kind="ExternalInput").ap()
    oh_d = nc.dram_tensor("oh", [P, 2], F32, kind="ExternalInput").ap()
    selT_d = nc.dram_tensor("selT", [2, P], F32, kind="ExternalInput").ap()
    invc_d = nc.dram_tensor("invc", [2, 1], F32, kind="ExternalInput").ap()

    rq_d = nc.dram_tensor("rq", [T, P], F32, kind="ExternalOutput").ap()
    mus_d = nc.dram_tensor("mus", [2, D], F32, kind="ExternalOutput").ap()
    lsds_d = nc.dram_tensor("lsds", [2, D], F32, kind="ExternalOutput").ap()

    with tile.TileContext(nc) as tc:
        with (
            tc.tile_pool(name="consts", bufs=1) as cp,
            tc.tile_pool(name="stream", bufs=2) as sp,
            tc.tile_pool(name="zpool", bufs=6) as zp,
            tc.tile_pool(name="coef", bufs=3) as cfp,
            tc.tile_pool(name="work", bufs=2) as wp,
            tc.tile_pool(name="acc", bufs=1, space="PSUM") as pp,
            tc.tile_pool(name="bcast", bufs=2, space="PSUM") as pbp,
            tc.tile_pool(name="dram", bufs=1, space="DRAM") as dp,
        ):
            oh = cp.tile([P, 2], F32)
            nc.sync.dma_start(oh, oh_d)
            selT = cp.tile([2, P], F32)
            nc.sync.dma_start(selT, selT_d)
            invc = cp.tile([2, 1], F32)
            nc.sync.dma_start(invc, invc_d)

            # ---- phase 1: per-class partial sums of mean / log_sd ----
            # Row tiles are summed position-wise on the VectorEngine as they
            # stream in (fp32 matmul on PE is 4 cyc/row and would dominate);
            # the one-hot matmul then reduces just the final [128, D] tile.
            accm = cp.tile([P, D], F32)
            accl = cp.tile([P, D], F32)
            for t in range(T):
                mt = sp.tile([P, D], F32, tag="m", name=f"m{t}")
                nc.sync.dma_start(mt, mean_d[t * P:(t + 1) * P, :])
                lt = sp.tile([P, D], F32, tag="l", name=f"l{t}")
                nc.sync.dma_start(lt, lsd_d[t * P:(t + 1) * P, :])
                if t == 0:
                    nc.scalar.copy(accm, mt)
                    nc.scalar.copy(accl, lt)
                else:
                    nc.vector.tensor_add(accm, accm, mt)
                    nc.vector.tensor_add(accl, accl, lt)

            # PE output base partition must be 0/32/64: mean sums live at
            # partitions 0-1, log_sd sums at partitions 32-33.
            ps = pp.tile([34, D], F32)
            for j in range(NCH):
                cs = slice(j * CH, (j + 1) * CH)
                nc.tensor.matmul(ps[0:2, cs], oh, accm[:, cs], start=True, stop=True)
                nc.tensor.matmul(ps[32:34, cs], oh, accl[:, cs], start=True, stop=True)

            # ---- all-reduce the partials across the 8 cores ----
            # cc layout [2, 2*D]: row c = [sum_mean_c (D) | sum_lsd_c (D)].
            # PSUM partials bounce through small SBUF tiles into DRAM.
            cc_in = dp.tile([2, 2 * D], F32)
            cc_out = dp.tile([2, 2 * D], F32, addr_space="Shared")
            Sm = cfp.tile([2, D], F32, tag="c", name="Sm")
            nc.scalar.copy(Sm, ps[0:2, :])
            nc.sync.dma_start(cc_in[:, 0:D], Sm)
            Sl = cfp.tile([2, D], F32, tag="c", name="Sl")
            nc.vector.tensor_copy(Sl, ps[32:34, :])
            nc.sync.dma_start(cc_in[:, D:2 * D], Sl)
            nc.gpsimd.collective_compute(
                "AllReduce",
                mybir.AluOpType.add,
                replica_groups=[list(range(N_CORES))],
                ins=[cc_in.opt()],
                outs=[cc_out.opt()],
            )
            Gm = cfp.tile([2, D], F32, tag="c", name="Gm")
            nc.sync.dma_start(Gm, cc_out[:, 0:D])
            Gl = cfp.tile([2, D], F32, tag="c", name="Gl")
            nc.sync.dma_start(Gl, cc_out[:, D:2 * D])

            # ---- coefficients ----
            Mm = cfp.tile([2, D], F32, tag="c", name="Mm")  # mus
            nc.vector.tensor_scalar_mul(Mm, Gm, invc)
            nc.sync.dma_start(mus_d, Mm)
            Ml = cfp.tile([2, D], F32, tag="c", name="Ml")  # lsds
            nc.vector.tensor_scalar_mul(Ml, Gl, invc)
            nc.sync.dma_start(lsds_d, Ml)
            IV = cfp.tile([2, D], F32, tag="c", name="IV")  # exp(-2*lsd)
            nc.scalar.activation(
                IV, Ml, mybir.ActivationFunctionType.Exp, scale=-2.0
            )
            SB = cfp.tile([2, D], F32, tag="c", name="SB")  # sqrt(0.5*exp(-2*lsd))
            nc.scalar.activation(
                SB, IV, mybir.ActivationFunctionType.Sqrt, scale=0.5
            )

            # Broadcast per-class rows to the 128-row class pattern via PE:
            # out[p, :] = coef[class(p), :]   (selT is the one-hot pattern^T)
            MU_bc = cp.tile([P, D], F32)
            SB_bc = cp.tile([P, D], F32)
            for j in range(NCH):
                cs = slice(j * CH, (j + 1) * CH)
                pm = pbp.tile([P, CH], F32, tag="bc", name=f"pm{j}")
                nc.tensor.matmul(pm, selT, Mm[:, cs], start=True, stop=True)
                nc.scalar.copy(MU_bc[:, cs], pm)
                pb = pbp.tile([P, CH], F32, tag="bc", name=f"pb{j}")
                nc.tensor.matmul(pb, selT, SB[:, cs], start=True, stop=True)
                nc.scalar.copy(SB_bc[:, cs], pb)

            # ---- phase 2: rq[n] = sum_d ((z - mu_sel) * sb_sel)^2 ----
            # All elementwise steps run in place on the z tile.
            for t in range(T):
                zt = zp.tile([P, D], F32, tag="z", name=f"z{t}")
                nc.scalar.dma_start(zt, z_d[t * P:(t + 1) * P, :])
                nc.vector.tensor_sub(zt, zt, MU_bc)
                nc.vector.tensor_mul(zt, zt, SB_bc)
                rq = wp.tile([P, 1], F32, tag="rq", name=f"rq{t}")
                nc.scalar.activation(
                    zt, zt, mybir.ActivationFunctionType.Square, accum_out=rq
                )
                nc.sync.dma_start(rq_d[t, :], rq[:, 0])

    nc.compile()
    return nc


def kernel(z, mean, log_sd, logdet, target):
    global LAST_RESULTS, _CACHED_NC

    z = np.ascontiguousarray(np.asarray(z, dtype=np.float32))
    mean = np.ascontiguousarray(np.asarray(mean, dtype=np.float32))
    log_sd = np.ascontiguousarray(np.asarray(log_sd, dtype=np.float32))
    logdet64 = np.asarray(logdet, dtype=np.float64)
    tgt = np.asarray(target).astype(np.int64)
    n, d = z.shape
    assert (n, d) == (N, D), f"kernel hardcoded for {(N, D)}, got {(n, d)}"

    # The device kernel assumes every 128-row tile has the same class pattern
    # (true for the arange%2 labels). Otherwise interleave the (balanced)
    # classes host-side and un-permute logp afterwards.
    pat = tgt[:P]
    perm = None
    tgt_dev = tgt
    if not (tgt.reshape(-1, P) == pat[None, :]).all():
        idx0 = np.where(tgt == 0)[0]
        idx1 = np.where(tgt == 1)[0]
        assert len(idx0) == len(idx1), "fallback layout needs balanced classes"
        perm = np.empty(n, dtype=np.int64)
        perm[0::2] = idx0
        perm[1::2] = idx1
        z, mean, log_sd = z[perm], mean[perm], log_sd[perm]
        tgt_dev = tgt[perm]
        pat = tgt_dev[:P]

    counts = np.array([(tgt == 0).sum(), (tgt == 1).sum()], dtype=np.float64)
    patf = pat.astype(np.float32)
    oh_np = np.ascontiguousarray(np.stack([1.0 - patf, patf], axis=1))  # [P, 2]
    selT_np = np.ascontiguousarray(oh_np.T)  # [2, P]
    invc_np = (1.0 / counts).astype(np.float32).reshape(2, 1)

    if _CACHED_NC is None:
        _CACHED_NC = _build_nc()
    nc = _CACHED_NC

    in_maps = [
        {
            "z": z[i * NL:(i + 1) * NL],
            "mean": mean[i * NL:(i + 1) * NL],
            "lsd": log_sd[i * NL:(i + 1) * NL],
            "oh": oh_np,
            "selT": selT_np,
            "invc": invc_np,
        }
        for i in range(N_CORES)
    ]
    res = run_bass_kernel_spmd(
        nc, in_maps, core_ids=list(range(N_CORES)), trace=TRACE
    )
    LAST_RESULTS = res

    rq = np.concatenate(
        [np.asarray(res.results[i]["rq"]).reshape(-1) for i in range(N_CORES)]
    )
    mus = np.asarray(res.results[0]["mus"])
    lsds = np.asarray(res.results[0]["lsds"])

    # logp[n] = C2[t_n] - rq[n];  C2[c] = -0.5*log(2pi)*D - sum_d lsds[c, d]
    c2 = -0.5 * LOG_2PI * D - lsds.astype(np.float64).sum(axis=1)
    logp = (c2[tgt_dev] - rq.astype(np.float64)).astype(np.float32)
    if perm is not None:
        inv = np.empty_like(perm)
        inv[perm] = np.arange(n)
        logp = logp[inv]

    logp64 = logp.astype(np.float64)
    lp_tot = np.array(
        [logp64[tgt == 0].sum() / counts[0], logp64[tgt == 1].sum() / counts[1]]
    )
    ld_tot = np.array(
        [logdet64[tgt == 0].sum() / counts[0], logdet64[tgt == 1].sum() / counts[1]]
    )
    prior_logprob = np.float32((lp_tot + ld_tot).mean())
    log_p_total = lp_tot.astype(np.float32)

    return prior_logprob, mus, lsds, logp, log_p_total


# revision 8
# speedup vs baseline: 1.3812x; 1.3433x over previous
"""Trainium2 Bass kernel for the 2-class Gaussian prior log-prob loss.

Reference math (N=8192 samples, D=3072 dims, 2 balanced classes):
    mus[c]  = mean over rows of class c of `mean`
    lsds[c] = mean over rows of class c of `log_sd`
    logp[n] = sum_d [ -0.5*log(2pi) - lsds[t_n,d]
                      - 0.5*(z[n,d]-mus[t_n,d])^2 * exp(-2*lsds[t_n,d]) ]
    log_p_total[c] = class-mean of logp;  prior = mean(log_p_total + logdet_total)

Distribution: COLUMN-parallel — core i owns d-slice [i*384, (i+1)*384) of all
8192 rows. The per-class mean reduction runs over N, which is fully local in
this layout, so no collective is needed at all: each core produces its slice
of mus/lsds exactly, plus a per-sample partial
    rq_i[n] = sum_{d in slice_i} ((z[n,d]-mus[t_n,d]) * sqrt(0.5*e^-2*lsds))^2
and the host adds the 8 partial vectors (8 x [8192] floats), applies
    logp[n] = C2[t_n] - sum_i rq_i[n],  C2[c] = -0.5*log(2pi)*D - sum_d lsds[c,d]
and finishes the tiny class-level reductions.

On-core schedule: mean/log_sd stream in as [128, 2*4*384] packed tiles (4
row-blocks side by side, mean|log_sd concatenated) and are position-wise
summed on the VectorEngine; a one-hot matmul turns the folded [128, 768]
accumulator into per-class sums. Phase 2 runs in place on packed z tiles:
DVE subtract/multiply, then ScalarE Square with per-row-block accumulate.
"""

import numpy as np

import concourse.bass as bass
import concourse.bacc as bacc
import concourse.mybir as mybir
import concourse.tile as tile
from concourse.bass_utils import run_bass_kernel_spmd

LOG_2PI = float(np.log(2.0 * np.pi))

N_CORES = 8
N = 8192
D = 3072
W = D // N_CORES           # columns per core (384)
P = 128                    # SBUF partitions
B = 4                      # row-blocks packed per SBUF tile
RT = N // P                # 64 row blocks
T = RT // B                # 16 packed tiles
PW = B * W                 # packed tile width (1536)

F32 = mybir.dt.float32

# Set by test.py to capture a hardware trace; last BassKernelResults lands in
# LAST_RESULTS for exec-time inspection.
TRACE = False
LAST_RESULTS = None

_CACHED_NC = None


def _build_nc():
    nc = bacc.Bacc(
        "TRN2",
        target_bir_lowering=False,
        debug=False,
        num_devices=N_CORES,
    )

    z_d = nc.dram_tensor("z", [N, W], F32, kind="ExternalInput").ap()
    mean_d = nc.dram_tensor("mean", [N, W], F32, kind="ExternalInput").ap()
    lsd_d = nc.dram_tensor("lsd", [N, W], F32, kind="ExternalInput").ap()
    oh_d = nc.dram_tensor("oh", [P, 2], F32, kind="ExternalInput").ap()
    selT_d = nc.dram_tensor("selT", [2, P], F32, kind="ExternalInput").ap()
    invc_d = nc.dram_tensor("invc", [2, 1], F32, kind="ExternalInput").ap()

    rq_d = nc.dram_tensor("rq", [P, RT], F32, kind="ExternalOutput").ap()
    mus_d = nc.dram_tensor("mus", [2, W], F32, kind="ExternalOutput").ap()
    lsds_d = nc.dram_tensor("lsds", [2, W], F32, kind="ExternalOutput").ap()

    # packed-tile source view: partition p, block b -> DRAM row t*B*P + b*P + p
    def packed(dram, t):
        return dram[t * B * P:(t + 1) * B * P, :].rearrange(
            "(b p) w -> p b w", b=B
        )

    with tile.TileContext(nc) as tc:
        with (
            tc.tile_pool(name="consts", bufs=1) as cp,
            tc.tile_pool(name="stream", bufs=3) as sp,
            tc.tile_pool(name="zpool", bufs=16) as zp,
            tc.tile_pool(name="coef", bufs=2) as cfp,
            tc.tile_pool(name="acc", bufs=1, space="PSUM") as pp,
            tc.tile_pool(name="bcast", bufs=2, space="PSUM") as pbp,
        ):
            oh = cp.tile([P, 2], F32)
            nc.sync.dma_start(oh, oh_d)
            selT = cp.tile([2, P], F32)
            nc.sync.dma_start(selT, selT_d)
            invc = cp.tile([2, 1], F32)
            nc.sync.dma_start(invc, invc_d)

            # ---- phase 1: position-wise sum of all row blocks ----
            # acc packs mean in [:, 0:PW] and log_sd in [:, PW:2*PW].
            acc = cp.tile([P, 2 * PW], F32)
            for t in range(T):
                mlt = sp.tile([P, 2 * PW], F32, tag="ml", name=f"ml{t}")
                nc.sync.dma_start(
                    mlt[:, 0:PW].rearrange("p (b w) -> p b w", b=B),
                    packed(mean_d, t),
                )
                nc.sync.dma_start(
                    mlt[:, PW:2 * PW].rearrange("p (b w) -> p b w", b=B),
                    packed(lsd_d, t),
                )
                if t == 0:
                    nc.scalar.copy(acc, mlt)
                else:
                    nc.vector.tensor_add(acc, acc, mlt)

            # fold the B row-blocks: [P, (2, B, W)] -> [P, (2, W)]
            accv = acc.rearrange("p (s b w) -> p s b w", s=2, b=B)
            fold = cp.tile([P, 2 * W], F32)
            foldv = fold.rearrange("p (s w) -> p s w", s=2)
            tmp = cp.tile([P, 2 * W], F32)
            tmpv = tmp.rearrange("p (s w) -> p s w", s=2)
            nc.vector.tensor_add(foldv, accv[:, :, 0, :], accv[:, :, 1, :])
            nc.vector.tensor_add(tmpv, accv[:, :, 2, :], accv[:, :, 3, :])
            nc.vector.tensor_add(fold, fold, tmp)

            # one-hot matmul: per-class sums [2, 2*W] (mean | lsd)
            ps = pp.tile([2, 2 * W], F32)
            nc.tensor.matmul(ps[:, 0:W], oh, fold[:, 0:W], start=True, stop=True)
            nc.tensor.matmul(
                ps[:, W:2 * W], oh, fold[:, W:2 * W], start=True, stop=True
            )

            # ---- coefficients (all local: no collective needed) ----
            M = cfp.tile([2, 2 * W], F32, tag="c", name="M")
            nc.vector.tensor_scalar_mul(M, ps, invc)  # [mu | lsd] slice means
            nc.sync.dma_start(mus_d, M[:, 0:W])
            nc.sync.dma_start(lsds_d, M[:, W:2 * W])
            IV = cfp.tile([2, W], F32, tag="c2", name="IV")  # exp(-2*lsd)
            nc.scalar.activation(
                IV, M[:, W:2 * W], mybir.ActivationFunctionType.Exp, scale=-2.0
            )
            SBc = cfp.tile([2, W], F32, tag="c2", name="SBc")
            nc.scalar.activation(
                SBc, IV, mybir.ActivationFunctionType.Sqrt, scale=0.5
            )

            # Broadcast class rows to the 128-row pattern, replicated B times:
            # MU4/SB4[p, b*W+w] = coef[class(p), w]
            MU4 = cp.tile([P, PW], F32)
            SB4 = cp.tile([P, PW], F32)
            pmu = pbp.tile([P, W], F32, tag="bc", name="pmu")
            nc.tensor.matmul(pmu, selT, M[:, 0:W], start=True, stop=True)
            psb = pbp.tile([P, W], F32, tag="bc", name="psb")
            nc.tensor.matmul(psb, selT, SBc, start=True, stop=True)
            for b in range(B):
                cs = slice(b * W, (b + 1) * W)
                nc.scalar.copy(MU4[:, cs], pmu)
                nc.vector.tensor_copy(SB4[:, cs], psb)

            # ---- phase 2: rq partials, in place on packed z tiles ----
            rq_sb = cp.tile([P, RT], F32)
            for t in range(T):
                zt = zp.tile([P, PW], F32, tag="z", name=f"z{t}")
                nc.scalar.dma_start(
                    zt.rearrange("p (b w) -> p b w", b=B), packed(z_d, t)
                )
                nc.vector.tensor_sub(zt, zt, MU4)
                nc.vector.tensor_mul(zt, zt, SB4)
                for b in range(B):
                    cs = slice(b * W, (b + 1) * W)
                    nc.scalar.activation(
                        zt[:, cs], zt[:, cs],
                        mybir.ActivationFunctionType.Square,
                        accum_out=rq_sb[:, t * B + b:t * B + b + 1],
                    )
            nc.sync.dma_start(rq_d, rq_sb)

    nc.compile()
    return nc


def kernel(z, mean, log_sd, logdet, target):
    global LAST_RESULTS, _CACHED_NC

    z = np.asarray(z, dtype=np.float32)
    mean = np.asarray(mean, dtype=np.float32)
    log_sd = np.asarray(log_sd, dtype=np.float32)
    logdet64 = np.asarray(logdet, dtype=np.float64)
    tgt = np.asarray(target).astype(np.int64)
    n, d = z.shape
    assert (n, d) == (N, D), f"kernel hardcoded for {(N, D)}, got {(n, d)}"

    # The device kernel assumes every 128-row block has the same class pattern
    # (true for the arange%2 labels). Otherwise interleave the (balanced)
    # classes host-side and un-permute logp afterwards.
    pat = tgt[:P]
    perm = None
    tgt_dev = tgt
    if not (tgt.reshape(-1, P) == pat[None, :]).all():
        idx0 = np.where(tgt == 0)[0]
        idx1 = np.where(tgt == 1)[0]
        assert len(idx0) == len(idx1), "fallback layout needs balanced classes"
        perm = np.empty(n, dtype=np.int64)
        perm[0::2] = idx0
        perm[1::2] = idx1
        z, mean, log_sd = z[perm], mean[perm], log_sd[perm]
        tgt_dev = tgt[perm]
        pat = tgt_dev[:P]

    counts = np.array([(tgt == 0).sum(), (tgt == 1).sum()], dtype=np.float64)
    patf = pat.astype(np.float32)
    oh_np = np.ascontiguousarray(np.stack([1.0 - patf, patf], axis=1))  # [P, 2]
    selT_np = np.ascontiguousarray(oh_np.T)  # [2, P]
    invc_np = (1.0 / counts).astype(np.float32).reshape(2, 1)

    if _CACHED_NC is None:
        _CACHED_NC = _build_nc()
    nc = _CACHED_NC

    in_maps = [
        {
            "z": np.ascontiguousarray(z[:, i * W:(i + 1) * W]),
            "mean": np.ascontiguousarray(mean[:, i * W:(i + 1) * W]),
            "lsd": np.ascontiguousarray(log_sd[:, i * W:(i + 1) * W]),
            "oh": oh_np,
            "selT": selT_np,
            "invc": invc_np,
        }
        for i in range(N_CORES)
    ]
    res = run_bass_kernel_spmd(
        nc, in_maps, core_ids=list(range(N_CORES)), trace=TRACE
    )
    LAST_RESULTS = res

    # rq[p, b] holds the partial for sample n = b*128 + p on each core
    rq = np.zeros(N, dtype=np.float64)
    for i in range(N_CORES):
        rq += np.asarray(res.results[i]["rq"]).T.reshape(-1).astype(np.float64)
    mus = np.concatenate(
        [np.asarray(res.results[i]["mus"]) for i in range(N_CORES)], axis=1
    )
    lsds = np.concatenate(
        [np.asarray(res.results[i]["lsds"]) for i in range(N_CORES)], axis=1
    )

    # logp[n] = C2[t_n] - rq[n];  C2[c] = -0.5*log(2pi)*D - sum_d lsds[c, d]
    c2 = -0.5 * LOG_2PI * D - lsds.astype(np.float64).sum(axis=1)
    logp = (c2[tgt_dev] - rq).astype(np.float32)
    if perm is not None:
        inv = np.empty_like(perm)
        inv[perm] = np.arange(n)
        logp = logp[inv]

    logp64 = logp.astype(np.float64)
    lp_tot = np.array(
        [logp64[tgt == 0].sum() / counts[0], logp64[tgt == 1].sum() / counts[1]]
    )
    ld_tot = np.array(
        [logdet64[tgt == 0].sum() / counts[0], logdet64[tgt == 1].sum() / counts[1]]
    )
    prior_logprob = np.float32((lp_tot + ld_tot).mean())
    log_p_total = lp_tot.astype(np.float32)

    return prior_logprob, mus, lsds, logp, log_p_total


# revision 9
# speedup vs baseline: 1.4623x; 1.0587x over previous
"""Trainium2 Bass kernel for the 2-class Gaussian prior log-prob loss.

Reference math (N=8192 samples, D=3072 dims, 2 balanced classes):
    mus[c]  = mean over rows of class c of `mean`
    lsds[c] = mean over rows of class c of `log_sd`
    logp[n] = sum_d [ -0.5*log(2pi) - lsds[t_n,d]
                      - 0.5*(z[n,d]-mus[t_n,d])^2 * exp(-2*lsds[t_n,d]) ]
    log_p_total[c] = class-mean of logp;  prior = mean(log_p_total + logdet_total)

Distribution: COLUMN-parallel — core i owns d-slice [i*384, (i+1)*384) of all
8192 rows. The per-class mean reduction runs over N, which is fully local in
this layout, so no collective is needed at all: each core produces its slice
of mus/lsds exactly, plus a per-sample partial
    rq_i[n] = sum_{d in slice_i} ((z[n,d]-mus[t_n,d]) * sqrt(0.5*e^-2*lsds))^2
and the host adds the 8 partial vectors (8 x [8192] floats), applies
    logp[n] = C2[t_n] - sum_i rq_i[n],  C2[c] = -0.5*log(2pi)*D - sum_d lsds[c,d]
and finishes the tiny class-level reductions.

On-core schedule: mean/log_sd stream in as [128, 2*4*384] packed tiles (4
row-blocks side by side, mean|log_sd concatenated) and are position-wise
summed on the VectorEngine; a one-hot matmul turns the folded [128, 768]
accumulator into per-class sums. Phase 2 runs in place on packed z tiles:
DVE subtract/multiply, then ScalarE Square with per-row-block accumulate.
"""

import numpy as np

import concourse.bass as bass
import concourse.bacc as bacc
import concourse.mybir as mybir
import concourse.tile as tile
from concourse.bass_utils import run_bass_kernel_spmd

LOG_2PI = float(np.log(2.0 * np.pi))

N_CORES = 8
N = 8192
D = 3072
W = D // N_CORES           # columns per core (384)
P = 128                    # SBUF partitions
B = 4                      # row-blocks packed per SBUF tile
RT = N // P                # 64 row blocks
T = RT // B                # 16 packed tiles
PW = B * W                 # packed tile width (1536)

F32 = mybir.dt.float32

# Set by test.py to capture a hardware trace; last BassKernelResults lands in
# LAST_RESULTS for exec-time inspection.
TRACE = False
LAST_RESULTS = None

_CACHED_NC = None


def _build_nc():
    nc = bacc.Bacc(
        "TRN2",
        target_bir_lowering=False,
        debug=False,
        num_devices=N_CORES,
    )

    z_d = nc.dram_tensor("z", [N, W], F32, kind="ExternalInput").ap()
    mean_d = nc.dram_tensor("mean", [N, W], F32, kind="ExternalInput").ap()
    lsd_d = nc.dram_tensor("lsd", [N, W], F32, kind="ExternalInput").ap()
    oh_d = nc.dram_tensor("oh", [P, 2], F32, kind="ExternalInput").ap()
    selT_d = nc.dram_tensor("selT", [2, P], F32, kind="ExternalInput").ap()
    invc_d = nc.dram_tensor("invc", [2, 1], F32, kind="ExternalInput").ap()

    rq_d = nc.dram_tensor("rq", [P, RT], F32, kind="ExternalOutput").ap()
    mus_d = nc.dram_tensor("mus", [2, W], F32, kind="ExternalOutput").ap()
    lsds_d = nc.dram_tensor("lsds", [2, W], F32, kind="ExternalOutput").ap()

    # packed-tile source view: partition p, block b -> DRAM row t*B*P + b*P + p
    def packed(dram, t):
        return dram[t * B * P:(t + 1) * B * P, :].rearrange(
            "(b p) w -> p b w", b=B
        )

    with tile.TileContext(nc) as tc:
        with (
            tc.tile_pool(name="consts", bufs=1) as cp,
            tc.tile_pool(name="stream", bufs=3) as sp,
            tc.tile_pool(name="zpool", bufs=16) as zp,
            tc.tile_pool(name="coef", bufs=2) as cfp,
            tc.tile_pool(name="acc", bufs=1, space="PSUM") as pp,
            tc.tile_pool(name="bcast", bufs=2, space="PSUM") as pbp,
        ):
            oh = cp.tile([P, 2], F32)
            nc.sync.dma_start(oh, oh_d)
            selT = cp.tile([2, P], F32)
            nc.sync.dma_start(selT, selT_d)
            invc = cp.tile([2, 1], F32)
            nc.sync.dma_start(invc, invc_d)

            # ---- phase 1: position-wise sum of all row blocks ----
            # acc packs mean in [:, 0:PW] and log_sd in [:, PW:2*PW].
            acc = cp.tile([P, 2 * PW], F32)
            for t in range(T):
                mlt = sp.tile([P, 2 * PW], F32, tag="ml", name=f"ml{t}")
                nc.sync.dma_start(
                    mlt[:, 0:PW].rearrange("p (b w) -> p b w", b=B),
                    packed(mean_d, t),
                )
                nc.sync.dma_start(
                    mlt[:, PW:2 * PW].rearrange("p (b w) -> p b w", b=B),
                    packed(lsd_d, t),
                )
                if t == 0:
                    nc.scalar.copy(acc, mlt)
                else:
                    nc.vector.tensor_add(acc, acc, mlt)

            # fold the B row-blocks: [P, (2, B, W)] -> [P, (2, W)]
            accv = acc.rearrange("p (s b w) -> p s b w", s=2, b=B)
            fold = cp.tile([P, 2 * W], F32)
            foldv = fold.rearrange("p (s w) -> p s w", s=2)
            tmp = cp.tile([P, 2 * W], F32)
            tmpv = tmp.rearrange("p (s w) -> p s w", s=2)
            nc.vector.tensor_add(foldv, accv[:, :, 0, :], accv[:, :, 1, :])
            nc.vector.tensor_add(tmpv, accv[:, :, 2, :], accv[:, :, 3, :])
            nc.vector.tensor_add(fold, fold, tmp)

            # one-hot matmuls: per-class sums. Each output goes to its own
            # PSUM bank (512 fp32) — a matmul output must not straddle banks.
            ps = pp.tile([2, 1024], F32)
            nc.tensor.matmul(ps[:, 0:W], oh, fold[:, 0:W], start=True, stop=True)
            nc.tensor.matmul(
                ps[:, 512:512 + W], oh, fold[:, W:2 * W], start=True, stop=True
            )

            # ---- coefficients (all local: no collective needed) ----
            M = cfp.tile([2, 2 * W], F32, tag="c", name="M")
            psv = ps.rearrange("c (s x) -> c s x", s=2)[:, :, 0:W]
            nc.vector.tensor_scalar_mul(
                M.rearrange("c (s w) -> c s w", s=2), psv, invc
            )  # [mu | lsd] slice means
            nc.sync.dma_start(mus_d, M[:, 0:W])
            nc.sync.dma_start(lsds_d, M[:, W:2 * W])
            IV = cfp.tile([2, W], F32, tag="c2", name="IV")  # exp(-2*lsd)
            nc.scalar.activation(
                IV, M[:, W:2 * W], mybir.ActivationFunctionType.Exp, scale=-2.0
            )
            SBc = cfp.tile([2, W], F32, tag="c2", name="SBc")
            nc.scalar.activation(
                SBc, IV, mybir.ActivationFunctionType.Sqrt, scale=0.5
            )

            # Broadcast class rows to the 128-row pattern, replicated B times:
            # MU4/SB4[p, b*W+w] = coef[class(p), w]
            MU4 = cp.tile([P, PW], F32)
            SB4 = cp.tile([P, PW], F32)
            pmu = pbp.tile([P, W], F32, tag="bc", name="pmu")
            nc.tensor.matmul(pmu, selT, M[:, 0:W], start=True, stop=True)
            psb = pbp.tile([P, W], F32, tag="bc", name="psb")
            nc.tensor.matmul(psb, selT, SBc, start=True, stop=True)
            for b in range(B):
                cs = slice(b * W, (b + 1) * W)
                nc.scalar.copy(MU4[:, cs], pmu)
                nc.vector.tensor_copy(SB4[:, cs], psb)

            # ---- phase 2: rq partials, in place on packed z tiles ----
            rq_sb = cp.tile([P, RT], F32)
            for t in range(T):
                zt = zp.tile([P, PW], F32, tag="z", name=f"z{t}")
                nc.scalar.dma_start(
                    zt.rearrange("p (b w) -> p b w", b=B), packed(z_d, t)
                )
                nc.vector.tensor_sub(zt, zt, MU4)
                nc.vector.tensor_mul(zt, zt, SB4)
                for b in range(B):
                    cs = slice(b * W, (b + 1) * W)
                    nc.scalar.activation(
                        zt[:, cs], zt[:, cs],
                        mybir.ActivationFunctionType.Square,
                        accum_out=rq_sb[:, t * B + b:t * B + b + 1],
                    )
            nc.sync.dma_start(rq_d, rq_sb)

    nc.compile()
    return nc


def kernel(z, mean, log_sd, logdet, target):
    global LAST_RESULTS, _CACHED_NC

    z = np.asarray(z, dtype=np.float32)
    mean = np.asarray(mean, dtype=np.float32)
    log_sd = np.asarray(log_sd, dtype=np.float32)
    logdet64 = np.asarray(logdet, dtype=np.float64)
    tgt = np.asarray(target).astype(np.int64)
    n, d = z.shape
    assert (n, d) == (N, D), f"kernel hardcoded for {(N, D)}, got {(n, d)}"

    # The device kernel assumes every 128-row block has the same class pattern
    # (true for the arange%2 labels). Otherwise interleave the (balanced)
    # classes host-side and un-permute logp afterwards.
    pat = tgt[:P]
    perm = None
    tgt_dev = tgt
    if not (tgt.reshape(-1, P) == pat[None, :]).all():
        idx0 = np.where(tgt == 0)[0]
        idx1 = np.where(tgt == 1)[0]
        assert len(idx0) == len(idx1), "fallback layout needs balanced classes"
        perm = np.empty(n, dtype=np.int64)
        perm[0::2] = idx0
        perm[1::2] = idx1
        z, mean, log_sd = z[perm], mean[perm], log_sd[perm]
        tgt_dev = tgt[perm]
        pat = tgt_dev[:P]

    counts = np.array([(tgt == 0).sum(), (tgt == 1).sum()], dtype=np.float64)
    patf = pat.astype(np.float32)
    oh_np = np.ascontiguousarray(np.stack([1.0 - patf, patf], axis=1))  # [P, 2]
    selT_np = np.ascontiguousarray(oh_np.T)  # [2, P]
    invc_np = (1.0 / counts).astype(np.float32).reshape(2, 1)

    if _CACHED_NC is None:
        _CACHED_NC = _build_nc()
    nc = _CACHED_NC

    in_maps = [
        {
            "z": np.ascontiguousarray(z[:, i * W:(i + 1) * W]),
            "mean": np.ascontiguousarray(mean[:, i * W:(i + 1) * W]),
            "lsd": np.ascontiguousarray(log_sd[:, i * W:(i + 1) * W]),
            "oh": oh_np,
            "selT": selT_np,
            "invc": invc_np,
        }
        for i in range(N_CORES)
    ]
    res = run_bass_kernel_spmd(
        nc, in_maps, core_ids=list(range(N_CORES)), trace=TRACE
    )
    LAST_RESULTS = res

    # rq[p, b] holds the partial for sample n = b*128 + p on each core
    rq = np.zeros(N, dtype=np.float64)
    for i in range(N_CORES):
        rq += np.asarray(res.results[i]["rq"]).T.reshape(-1).astype(np.float64)
    mus = np.concatenate(
        [np.asarray(res.results[i]["mus"]) for i in range(N_CORES)], axis=1
    )
    lsds = np.concatenate(
        [np.asarray(res.results[i]["lsds"]) for i in range(N_CORES)], axis=1
    )

    # logp[n] = C2[t_n] - rq[n];  C2[c] = -0.5*log(2pi)*D - sum_d lsds[c, d]
    c2 = -0.5 * LOG_2PI * D - lsds.astype(np.float64).sum(axis=1)
    logp = (c2[tgt_dev] - rq).astype(np.float32)
    if perm is not None:
        inv = np.empty_like(perm)
        inv[perm] = np.arange(n)
        logp = logp[inv]

    logp64 = logp.astype(np.float64)
    lp_tot = np.array(
        [logp64[tgt == 0].sum() / counts[0], logp64[tgt == 1].sum() / counts[1]]
    )
    ld_tot = np.array(
        [logdet64[tgt == 0].sum() / counts[0], logdet64[tgt == 1].sum() / counts[1]]
    )
    prior_logprob = np.float32((lp_tot + ld_tot).mean())
    log_p_total = lp_tot.astype(np.float32)

    return prior_logprob, mus, lsds, logp, log_p_total


# revision 10
# speedup vs baseline: 1.7719x; 1.2117x over previous
"""Trainium2 Bass kernel for the 2-class Gaussian prior log-prob loss.

Reference math (N=8192 samples, D=3072 dims, 2 balanced classes):
    mus[c]  = mean over rows of class c of `mean`
    lsds[c] = mean over rows of class c of `log_sd`
    logp[n] = sum_d [ -0.5*log(2pi) - lsds[t_n,d]
                      - 0.5*(z[n,d]-mus[t_n,d])^2 * exp(-2*lsds[t_n,d]) ]
    log_p_total[c] = class-mean of logp;  prior = mean(log_p_total + logdet_total)

Distribution: COLUMN-parallel — core i owns d-slice [i*384, (i+1)*384) of all
8192 rows. The per-class mean reduction runs over N, which is fully local in
this layout, so no collective is needed at all: each core produces its slice
of mus/lsds exactly, plus a per-sample partial
    rq_i[n] = sum_{d in slice_i} ((z[n,d]-mus[t_n,d]) * sqrt(0.5*e^-2*lsds))^2
and the host adds the 8 partial vectors (8 x [8192] floats), applies
    logp[n] = C2[t_n] - sum_i rq_i[n],  C2[c] = -0.5*log(2pi)*D - sum_d lsds[c,d]
and finishes the tiny class-level reductions.

On-core schedule: mean/log_sd stream in as [128, 2*4*384] packed tiles (4
row-blocks side by side, mean|log_sd concatenated) and are position-wise
summed on the VectorEngine; a one-hot matmul turns the folded [128, 768]
accumulator into per-class sums. Phase 2 runs in place on packed z tiles:
DVE subtract/multiply, then ScalarE Square with per-row-block accumulate.
"""

import numpy as np

import concourse.bass as bass
import concourse.bacc as bacc
import concourse.mybir as mybir
import concourse.tile as tile
from concourse.bass_utils import run_bass_kernel_spmd

LOG_2PI = float(np.log(2.0 * np.pi))

N_CORES = 8
N = 8192
D = 3072
W = D // N_CORES           # columns per core (384)
P = 128                    # SBUF partitions
B = 4                      # row-blocks packed per SBUF tile
RT = N // P                # 64 row blocks
T = RT // B                # 16 packed tiles
PW = B * W                 # packed tile width (1536)

F32 = mybir.dt.float32

# Set by test.py to capture a hardware trace; last BassKernelResults lands in
# LAST_RESULTS for exec-time inspection.
TRACE = False
LAST_RESULTS = None

_CACHED_NC = None


def _build_nc():
    nc = bacc.Bacc(
        "TRN2",
        target_bir_lowering=False,
        debug=False,
        num_devices=N_CORES,
    )

    z_d = nc.dram_tensor("z", [N, W], F32, kind="ExternalInput").ap()
    mean_d = nc.dram_tensor("mean", [N, W], F32, kind="ExternalInput").ap()
    lsd_d = nc.dram_tensor("lsd", [N, W], F32, kind="ExternalInput").ap()
    oh_d = nc.dram_tensor("oh", [P, 2], F32, kind="ExternalInput").ap()
    selT_d = nc.dram_tensor("selT", [2, P], F32, kind="ExternalInput").ap()
    invc_d = nc.dram_tensor("invc", [2, 1], F32, kind="ExternalInput").ap()

    rq_d = nc.dram_tensor("rq", [P, RT], F32, kind="ExternalOutput").ap()
    mus_d = nc.dram_tensor("mus", [2, W], F32, kind="ExternalOutput").ap()
    lsds_d = nc.dram_tensor("lsds", [2, W], F32, kind="ExternalOutput").ap()

    # packed-tile source view: partition p, block b -> DRAM row t*B*P + b*P + p
    def packed(dram, t):
        return dram[t * B * P:(t + 1) * B * P, :].rearrange(
            "(b p) w -> p b w", b=B
        )

    with tile.TileContext(nc) as tc:
        with (
            tc.tile_pool(name="consts", bufs=1) as cp,
            tc.tile_pool(name="stream", bufs=3) as sp,
            tc.tile_pool(name="zpool", bufs=16) as zp,
            tc.tile_pool(name="coef", bufs=2) as cfp,
            tc.tile_pool(name="acc", bufs=1, space="PSUM") as pp,
            tc.tile_pool(name="bcast", bufs=2, space="PSUM") as pbp,
        ):
            oh = cp.tile([P, 2], F32)
            nc.scalar.dma_start(oh, oh_d)
            selT = cp.tile([2, P], F32)
            nc.scalar.dma_start(selT, selT_d)
            invc = cp.tile([2, 1], F32)
            nc.scalar.dma_start(invc, invc_d)

            # ---- phase 1: position-wise sum of all row blocks ----
            # acc packs mean in [:, 0:PW] and log_sd in [:, PW:2*PW].
            acc = cp.tile([P, 2 * PW], F32)
            for t in range(T):
                mlt = sp.tile([P, 2 * PW], F32, tag="ml", name=f"ml{t}")
                nc.sync.dma_start(
                    mlt[:, 0:PW].rearrange("p (b w) -> p b w", b=B),
                    packed(mean_d, t),
                )
                nc.sync.dma_start(
                    mlt[:, PW:2 * PW].rearrange("p (b w) -> p b w", b=B),
                    packed(lsd_d, t),
                )
                if t == 0:
                    nc.scalar.copy(acc, mlt)
                else:
                    nc.vector.tensor_add(acc, acc, mlt)

            # fold the B row-blocks: [P, (2, B, W)] -> [P, (2, W)]
            accv = acc.rearrange("p (s b w) -> p s b w", s=2, b=B)
            fold = cp.tile([P, 2 * W], F32)
            foldv = fold.rearrange("p (s w) -> p s w", s=2)
            tmp = cp.tile([P, 2 * W], F32)
            tmpv = tmp.rearrange("p (s w) -> p s w", s=2)
            nc.vector.tensor_add(foldv, accv[:, :, 0, :], accv[:, :, 1, :])
            nc.vector.tensor_add(tmpv, accv[:, :, 2, :], accv[:, :, 3, :])
            nc.vector.tensor_add(fold, fold, tmp)

            # one-hot matmuls: per-class sums. Each output goes to its own
            # PSUM bank (512 fp32) — a matmul output must not straddle banks.
            ps = pp.tile([2, 1024], F32)
            nc.tensor.matmul(ps[:, 0:W], oh, fold[:, 0:W], start=True, stop=True)
            nc.tensor.matmul(
                ps[:, 512:512 + W], oh, fold[:, W:2 * W], start=True, stop=True
            )

            # ---- coefficients (all local: no collective needed) ----
            M = cfp.tile([2, 2 * W], F32, tag="c", name="M")
            psv = ps.rearrange("c (s x) -> c s x", s=2)[:, :, 0:W]
            nc.vector.tensor_scalar_mul(
                M.rearrange("c (s w) -> c s w", s=2), psv, invc
            )  # [mu | lsd] slice means
            nc.scalar.dma_start(mus_d, M[:, 0:W])
            nc.scalar.dma_start(lsds_d, M[:, W:2 * W])
            IV = cfp.tile([2, W], F32, tag="c2", name="IV")  # exp(-2*lsd)
            nc.scalar.activation(
                IV, M[:, W:2 * W], mybir.ActivationFunctionType.Exp, scale=-2.0
            )
            SBc = cfp.tile([2, W], F32, tag="c2", name="SBc")
            nc.scalar.activation(
                SBc, IV, mybir.ActivationFunctionType.Sqrt, scale=0.5
            )

            # Broadcast class rows to the 128-row pattern, replicated B times:
            # MU4/SB4[p, b*W+w] = coef[class(p), w]
            MU4 = cp.tile([P, PW], F32)
            SB4 = cp.tile([P, PW], F32)
            pmu = pbp.tile([P, W], F32, tag="bc", name="pmu")
            nc.tensor.matmul(pmu, selT, M[:, 0:W], start=True, stop=True)
            psb = pbp.tile([P, W], F32, tag="bc", name="psb")
            nc.tensor.matmul(psb, selT, SBc, start=True, stop=True)
            for b in range(B):
                cs = slice(b * W, (b + 1) * W)
                nc.scalar.copy(MU4[:, cs], pmu)
                nc.vector.tensor_copy(SB4[:, cs], psb)

            # ---- phase 2: rq partials, in place on packed z tiles ----
            rq_sb = cp.tile([P, RT], F32)
            for t in range(T):
                zt = zp.tile([P, PW], F32, tag="z", name=f"z{t}")
                # same HWDGE ring as the mean/log_sd stream: ring FIFO keeps
                # the phase-1 stream strictly ahead of the z prefetch
                nc.sync.dma_start(
                    zt.rearrange("p (b w) -> p b w", b=B), packed(z_d, t)
                )
                nc.vector.tensor_sub(zt, zt, MU4)
                nc.vector.tensor_mul(zt, zt, SB4)
                for b in range(B):
                    cs = slice(b * W, (b + 1) * W)
                    nc.scalar.activation(
                        zt[:, cs], zt[:, cs],
                        mybir.ActivationFunctionType.Square,
                        accum_out=rq_sb[:, t * B + b:t * B + b + 1],
                    )
            nc.scalar.dma_start(rq_d, rq_sb)

    nc.compile()
    return nc


def kernel(z, mean, log_sd, logdet, target):
    global LAST_RESULTS, _CACHED_NC

    z = np.asarray(z, dtype=np.float32)
    mean = np.asarray(mean, dtype=np.float32)
    log_sd = np.asarray(log_sd, dtype=np.float32)
    logdet64 = np.asarray(logdet, dtype=np.float64)
    tgt = np.asarray(target).astype(np.int64)
    n, d = z.shape
    assert (n, d) == (N, D), f"kernel hardcoded for {(N, D)}, got {(n, d)}"

    # The device kernel assumes every 128-row block has the same class pattern
    # (true for the arange%2 labels). Otherwise interleave the (balanced)
    # classes host-side and un-permute logp afterwards.
    pat = tgt[:P]
    perm = None
    tgt_dev = tgt
    if not (tgt.reshape(-1, P) == pat[None, :]).all():
        idx0 = np.where(tgt == 0)[0]
        idx1 = np.where(tgt == 1)[0]
        assert len(idx0) == len(idx1), "fallback layout needs balanced classes"
        perm = np.empty(n, dtype=np.int64)
        perm[0::2] = idx0
        perm[1::2] = idx1
        z, mean, log_sd = z[perm], mean[perm], log_sd[perm]
        tgt_dev = tgt[perm]
        pat = tgt_dev[:P]

    counts = np.array([(tgt == 0).sum(), (tgt == 1).sum()], dtype=np.float64)
    patf = pat.astype(np.float32)
    oh_np = np.ascontiguousarray(np.stack([1.0 - patf, patf], axis=1))  # [P, 2]
    selT_np = np.ascontiguousarray(oh_np.T)  # [2, P]
    invc_np = (1.0 / counts).astype(np.float32).reshape(2, 1)

    if _CACHED_NC is None:
        _CACHED_NC = _build_nc()
    nc = _CACHED_NC

    in_maps = [
        {
            "z": np.ascontiguousarray(z[:, i * W:(i + 1) * W]),
            "mean": np.ascontiguousarray(mean[:, i * W:(i + 1) * W]),
            "lsd": np.ascontiguousarray(log_sd[:, i * W:(i + 1) * W]),
            "oh": oh_np,
            "selT": selT_np,
            "invc": invc_np,
        }
        for i in range(N_CORES)
    ]
    res = run_bass_kernel_spmd(
        nc, in_maps, core_ids=list(range(N_CORES)), trace=TRACE
    )
    LAST_RESULTS = res

    # rq[p, b] holds the partial for sample n = b*128 + p on each core
    rq = np.zeros(N, dtype=np.float64)
    for i in range(N_CORES):
        rq += np.asarray(res.results[i]["rq"]).T.reshape(-1).astype(np.float64)
    mus = np.concatenate(
        [np.asarray(res.results[i]["mus"]) for i in range(N_CORES)], axis=1
    )
    lsds = np.concatenate(
        [np.asarray(res.results[i]["lsds"]) for i in range(N_CORES)], axis=1
    )

    # logp[n] = C2[t_n] - rq[n];  C2[c] = -0.5*log(2pi)*D - sum_d lsds[c, d]
    c2 = -0.5 * LOG_2PI * D - lsds.astype(np.float64).sum(axis=1)
    logp = (c2[tgt_dev] - rq).astype(np.float32)
    if perm is not None:
        inv = np.empty_like(perm)
        inv[perm] = np.arange(n)
        logp = logp[inv]

    logp64 = logp.astype(np.float64)
    lp_tot = np.array(
        [logp64[tgt == 0].sum() / counts[0], logp64[tgt == 1].sum() / counts[1]]
    )
    ld_tot = np.array(
        [logdet64[tgt == 0].sum() / counts[0], logdet64[tgt == 1].sum() / counts[1]]
    )
    prior_logprob = np.float32((lp_tot + ld_tot).mean())
    log_p_total = lp_tot.astype(np.float32)

    return prior_logprob, mus, lsds, logp, log_p_total
